# revision 49
# baseline (speedup 1.0000x reference)
"""BERT self-attention (B=4, S=2048, H=768, 12 heads) on 8 NeuronCores.

Sharding: core c handles batch b=c//2, query-half qh=c%2 (1024 q rows).
K/V are computed for the full sequence on each core (duplicated across the
2 cores of a batch) so no collectives are needed. Matmul operands are fp16;
accumulation stays fp32 in PSUM.

Host-side prep: x arrives transposed+rotated AND pre-chunked (per-partition
contiguous arrays), weights pre-arranged [128, hc, cols] and split at the
column boundaries the kernel loads first — every input DMA is one
contiguous run per partition (~128 descriptors), so triggers cost ~0.3us
instead of 4-7us and the critical first-scores loads land in ~14us.

Emission is explicitly software-pipelined (the tile scheduler's reorder
window is small, so emission order ~ execution order per engine):
  - scores(kc+1) before ctx(kc); the next pass's scores(0) before this
    pass's last ctx -> the ACT exp stream never waits on the PE queue.
  - deferred work (K/Q proj, V tail, normalize, out-proj) is queued as
    closures and pumped at a fixed cadence into attention windows.
  - warmup matmuls + a dummy exp run during the input DMAs (HAM clock
    ramp + ACT exp-table prefetch), made load-bearing via ones = exp(0).
  - tail: out-proj mc=0..4 partial sums held in psum overlap the final
    reciprocal; only the mc=5 step waits on the last normalize; gelu is
    batched after one ACT table switch.
"""

import sys

sys.path.insert(0, "/opt/trn_rl_repo")

import numpy as np

import concourse.bass as bass
import concourse.tile as tile
import concourse.mybir as mybir

F16 = mybir.dt.float16
F32 = mybir.dt.float32
AF = mybir.ActivationFunctionType
ALU = mybir.AluOpType

S = 2048
SQ = 1024
H = 768
NH = 12
DH = 64
HC = H // 128  # 6
SC = S // 128  # 16
QC = SQ // 128  # 8
VW = DH + 1  # 65
QN = SQ // 512  # 2
NG = NH * QN  # 24


def split_sync_waits(nc, cap=1):
    """Walrus here rejects instructions carrying more than ~1 sync wait.
    Move excess waits onto same-engine NoOps inserted just before."""
    n = 0
    for b in nc.m.functions[0].blocks:
        out = []
        for inst in b.instructions:
            si = inst.sync_info
            waits = list(si.on_wait) if si is not None and si.on_wait else []
            if len(waits) > cap:
                extra, keep = waits[:-cap], waits[-cap:]
                for i in range(0, len(extra), cap):
                    nop = mybir.InstNoOp(
                        name=f"wsplit-{n}",
                        engine=inst.engine,
                        sync_info=mybir.SyncInfo(
                            on_wait=extra[i : i + cap], on_update=[]
                        ),
                    )
                    n += 1
                    out.append(nop)
                si.on_wait = keep
            out.append(inst)
        b.instructions[:] = out
    return n


def build_program():
    from collections import deque

    nc = bass.Bass()
    # x chunks: [128, ic, 512] per-partition-contiguous
    x0a = nc.declare_dram_parameter("x0a", [128, 3, 512], F16, isOutput=False)
    x0b = nc.declare_dram_parameter("x0b", [128, 3, 512], F16, isOutput=False)
    x1 = nc.declare_dram_parameter("x1", [128, HC, 512], F16, isOutput=False)
    x2 = nc.declare_dram_parameter("x2", [128, HC, 512], F16, isOutput=False)
    x3 = nc.declare_dram_parameter("x3", [128, HC, 512], F16, isOutput=False)
    ident_in = nc.declare_dram_parameter("ident", [128, 128], F16, isOutput=False)
    wka = nc.declare_dram_parameter("wka", [128, HC, 128], F16, isOutput=False)
    wkb = nc.declare_dram_parameter("wkb", [128, HC, 640], F16, isOutput=False)
    wqa = nc.declare_dram_parameter("wqa", [128, HC, 128], F16, isOutput=False)
    wqb = nc.declare_dram_parameter("wqb", [128, HC, 640], F16, isOutput=False)
    wva = nc.declare_dram_parameter("wva", [128, HC, 512], F16, isOutput=False)
    wvb = nc.declare_dram_parameter("wvb", [128, HC, 256], F16, isOutput=False)
    wop = nc.declare_dram_parameter("wop", [128, HC, H], F16, isOutput=False)
    bqf = nc.declare_dram_parameter("bqf", [H], F32, isOutput=False)
    bkf = nc.declare_dram_parameter("bkf", [H], F32, isOutput=False)
    bv16 = nc.declare_dram_parameter("bv16", [H], F16, isOutput=False)
    bo16 = nc.declare_dram_parameter("bo16", [H], F16, isOutput=False)
    out = nc.declare_dram_parameter("out", [SQ, H], F32, isOutput=True)

    with tile.TileContext(nc) as tc:
        from contextlib import ExitStack

        with ExitStack() as ctx:
            consts = ctx.enter_context(tc.tile_pool(name="consts", bufs=1))
            wpool = ctx.enter_context(tc.tile_pool(name="wpool", bufs=1))
            big = ctx.enter_context(tc.tile_pool(name="big", bufs=1))
            copystage = ctx.enter_context(tc.tile_pool(name="copystage", bufs=3))
            etpool = ctx.enter_context(tc.tile_pool(name="etpool", bufs=8))
            recpool = ctx.enter_context(tc.tile_pool(name="recpool", bufs=1))
            outstage = ctx.enter_context(tc.tile_pool(name="outstage", bufs=2))
            pp_mm = ctx.enter_context(
                tc.tile_pool(name="pp_mm", bufs=2, space="PSUM")
            )

            # ---- constants (fast queues; gpsimd preamble is ~7us) ----
            ident = consts.tile([128, 128], F16, tag="ident")
            nc.sync.dma_start(ident[:], ident_in[:])
            junk = consts.tile([128, 512], F32, tag="junk")
            nc.vector.memset(junk[:], 0.0)
            # ones via exp(0): ACT exp-table load becomes load-bearing+early
            ones16 = consts.tile([128, 512], F16, tag="ones16")
            nc.scalar.activation(ones16[:], junk[:], AF.Exp)

            # ---- HAM warmup, load-bearing: onesW = ident.T @ ones16 ----
            onesW = consts.tile([128, 512], F16, tag="onesW")
            with tc.tile_pool(name="pp_warm", bufs=1, space="PSUM") as pp_warm:
                warm = pp_warm.tile([128, 512], F32, tag="warm")
                for _ in range(32):
                    nc.tensor.matmul(
                        warm[:, 0:128],
                        ident[:],
                        ones16[:, 0:128],
                        start=True,
                        stop=True,
                    )
                nc.tensor.matmul(
                    warm[:], ident[:], ones16[:], start=True, stop=True
                )
                nc.vector.tensor_copy(onesW[:], warm[:])

            # ---- SBUF tiles for weights / x ----
            wka_sb = wpool.tile([128, HC, 128], F16, tag="wka")
            wkb_sb = wpool.tile([128, HC, 640], F16, tag="wkb")
            wqa_sb = wpool.tile([128, HC, 128], F16, tag="wqa")
            wqb_sb = wpool.tile([128, HC, 640], F16, tag="wqb")
            wva_sb = wpool.tile([128, HC, 512], F16, tag="wva")
            wvb_sb = wpool.tile([128, HC, 256], F16, tag="wvb")
            wo_sb = wpool.tile([128, HC, H], F16, tag="wo")
            bq_sb = wpool.tile([128, HC], F32, tag="bq")
            bk_sb = wpool.tile([128, HC], F32, tag="bk")
            bv_sb = wpool.tile([1, H], F16, tag="bv")
            bo_sb = wpool.tile([1, H], F16, tag="bo")
            # x as [128, chunk, ic, 512] so chunk loads are contiguous
            xTc = big.tile([128, 4, HC, 512], F16, tag="xTc")
            v_sb = big.tile([128, SC, NH * VW], F16, tag="v")
            v_heads = v_sb[:].rearrange("p s (h c) -> p s h c", c=VW)

            # ---- DMA queue plan. Within a queue transfers serialize, so
            # order IS the priority. Critical for first scores: x0a+x0b,
            # wka, wqa (~1.1MB). Everything else queues behind. ----
            # sync:   ident, x0a, x2, x3
            # scalar: x0b, wka, wqa, wva, wkb, wqb, wvb, wop
            # gpsimd: v-ones memset, x1, biases, tail memsets
            nc.gpsimd.memset(v_heads[:, :, :, DH], 1.0)
            nc.scalar.dma_start(wka_sb[:], wka[:])
            nc.scalar.dma_start(wqa_sb[:], wqa[:])
            # x chunk0 split per-ic: the K(0,sn0) accumulation chain
            # starts when ic0 lands (~1us) instead of the whole chunk
            for i in range(3):
                nc.sync.dma_start(xTc[:, 0, i, :], x0a[:, i, :])
            for i in range(3):
                nc.scalar.dma_start(xTc[:, 0, 3 + i, :], x0b[:, i, :])
            nc.gpsimd.dma_start(xTc[:, 1, 0:3, :], x1[:, 0:3, :])
            nc.sync.dma_start(xTc[:, 1, 3:6, :], x1[:, 3:6, :])
            nc.scalar.dma_start(wva_sb[:], wva[:])
            nc.sync.dma_start(xTc[:, 2, :, :], x2[:])
            nc.sync.dma_start(xTc[:, 3, :, :], x3[:])
            nc.gpsimd.dma_start(bq_sb[:], bqf.rearrange("(c p) -> p c", p=128))
            nc.gpsimd.dma_start(bk_sb[:], bkf.rearrange("(c p) -> p c", p=128))
            nc.gpsimd.dma_start(bv_sb[:], bv16[None, :])
            nc.gpsimd.dma_start(bo_sb[:], bo16[None, :])
            nc.scalar.dma_start(wkb_sb[:], wkb[:])
            nc.scalar.dma_start(wqb_sb[:], wqb[:])
            nc.scalar.dma_start(wvb_sb[:], wvb[:])
            nc.scalar.dma_start(wo_sb[:], wop[:])

            def kq_w(proj, ic, hc):
                if hc == 0:
                    return (wka_sb if proj == "k" else wqa_sb)[:, ic, :]
                t = wkb_sb if proj == "k" else wqb_sb
                return t[:, ic, (hc - 1) * 128 : hc * 128]

            def x_at(ic, s0, sw):
                """xTc slice for seq [s0, s0+sw); must stay in one chunk."""
                ch = s0 // 512
                o = s0 % 512
                return xTc[:, ch, ic, o : o + sw]

            # ---- bias broadcast across partitions (K=1 matmul on onesW) --
            bv_bc = wpool.tile([128, H], F32, tag="bv_bc")
            bo_bc = wpool.tile([128, H], F32, tag="bo_bc")

            def emit_bias_bc(bc, bsb, c0, cw):
                ps = pp_mm.tile([128, 512], F32, tag="pp_mm")
                nc.tensor.matmul(
                    ps[:, :cw],
                    onesW[0:1, 0:128],
                    bsb[:, c0 : c0 + cw],
                    start=True,
                    stop=True,
                )
                nc.vector.tensor_copy(bc[:, c0 : c0 + cw], ps[:, :cw])

            def emit_v_grp(sc, c0, cw):
                """V columns c0:c0+cw for seq chunk sc (c0 head-aligned:
                0:512 = heads 0-7, 512:768 = heads 8-11)."""
                ps = pp_mm.tile([128, 512], F32, tag="pp_mm")
                wsrc = wva_sb if c0 == 0 else wvb_sb
                for ic in range(HC):
                    nc.tensor.matmul(
                        ps[:, :cw],
                        x_at(ic, sc * 128, 128),
                        wsrc[:, ic, :],
                        start=(ic == 0),
                        stop=(ic == HC - 1),
                    )
                h0 = c0 // DH
                nhh = cw // DH
                nc.vector.scalar_tensor_tensor(
                    v_heads[:, sc, h0 : h0 + nhh, 0:DH],
                    ps[:, :cw].rearrange("p (h c) -> p h c", c=DH),
                    1.0,
                    bv_bc[:, c0 : c0 + cw].rearrange("p (h c) -> p h c", c=DH),
                    ALU.mult,
                    ALU.add,
                )

            pp_s = ctx.enter_context(tc.tile_pool(name="pp_s", bufs=2, space="PSUM"))
            pp_c = ctx.enter_context(tc.tile_pool(name="pp_c", bufs=2, space="PSUM"))

            out_t = out.rearrange("(n p) h -> n p h", p=128)

            kT = big.tile([128, HC, S], F16, tag="kT")
            qT = big.tile([128, HC, SQ], F16, tag="qT")
            ctxU = big.tile([128, HC, SQ], F16, tag="ctxU")
            stash = big.tile([128, QC, H], F32, tag="stash")
            # group row map: bases must be 0/32/64/96 for reciprocal
            # batches: rows 0-11 = heads 0-5 both qn; 32-35 = h6-9 qn0;
            # 64-67 = h6-9 qn1; 96-97 = h10-11 qn1 (tail); 98-99 = h10-11
            # qn0 (computed in the [96:100) batch after (5,0); its rows
            # 96/97 are then still padding, harmless).
            NR = 100
            rows_sb = big.tile([NR, 512], F32, tag="rows")
            recip16 = big.tile([NR, 512], F16, tag="recip16")
            nc.gpsimd.memset(recip16[:], 0.0)
            nc.gpsimd.memset(rows_sb[:], 1.0)

            def grow(g):
                h, qn = g // QN, g % QN
                if h < NH // 2:
                    return g
                if qn == 0:
                    return 32 + (h - NH // 2) if h < 10 else 98 + (h - 10)
                return 64 + (h - NH // 2) if h < 10 else 96 + (h - 10)

            def emit_kq_grp(proj, hc, sn):
                """One (proj, hc, sn) group: 6 accum MMs + bias add."""
                ps = pp_mm.tile([128, 512], F32, tag="pp_mm")
                for ic in range(HC):
                    nc.tensor.matmul(
                        ps[:],
                        kq_w(proj, ic, hc),
                        xTc[:, sn, ic, :],
                        start=(ic == 0),
                        stop=(ic == HC - 1),
                    )
                if proj == "k":
                    nc.vector.tensor_scalar_add(
                        kT[:, hc, sn * 512 : (sn + 1) * 512],
                        ps[:],
                        bk_sb[:, hc : hc + 1],
                    )
                else:
                    nc.vector.tensor_scalar(
                        qT[:, hc, sn * 512 : (sn + 1) * 512],
                        ps[:],
                        bq_sb[:, hc : hc + 1],
                        0.125,
                        ALU.add,
                        ALU.mult,
                    )

            def kq_closures(hc, skip_first=False):
                cl = []
                if not skip_first:
                    cl.append(lambda hc=hc: emit_kq_grp("q", hc, 0))
                for sn in range(4):
                    if skip_first and sn == 0:
                        continue
                    cl.append(lambda hc=hc, sn=sn: emit_kq_grp("k", hc, sn))
                cl.append(lambda hc=hc: emit_kq_grp("q", hc, 1))
                return cl

            def normalize_group(h, qn):
                hb = (h % 2) * 64
                r = grow(h * QN + qn)
                pb = pp_mm.tile([128, 512], F32, tag="pp_mm")
                nc.tensor.matmul(
                    pb[hb : hb + 64, :],
                    ident[0:NR, r : r + 1].to_broadcast([NR, 64]),
                    recip16[:],
                    start=True,
                    stop=True,
                )
                sl = ctxU[hb : hb + 64, h // 2, qn * 512 : (qn + 1) * 512]
                nc.vector.tensor_tensor(sl, sl, pb[hb : hb + 64, :], ALU.mult)

            def emit_out_grp(qc, c0, cw):
                ps = pp_mm.tile([128, 512], F32, tag="pp_mm")
                for mc in range(HC):
                    nc.tensor.matmul(
                        ps[:, :cw],
                        ctxU[:, mc, qc * 128 : (qc + 1) * 128],
                        wo_sb[:, mc, c0 : c0 + cw],
                        start=(mc == 0),
                        stop=(mc == HC - 1),
                    )
                nc.vector.tensor_tensor(
                    stash[:, qc, c0 : c0 + cw],
                    ps[:, :cw],
                    bo_bc[:, c0 : c0 + cw],
                    ALU.add,
                )

            def emit_out_tail(qc):
                ost = outstage.tile([128, H], F32, tag="ost")
                nc.scalar.activation(ost[:], stash[:, qc, :], AF.Gelu)
                q = nc.sync if qc % 2 == 0 else nc.scalar
                q.dma_start(out_t[qc][:, :], ost[:])

            # ---- deferred-work queue, pumped into attention windows ----
            extra = deque()

            def pump(n=1):
                for _ in range(n):
                    if not extra:
                        return
                    extra.popleft()()

            def q_recip(lo, hi):
                def go():
                    rec = recpool.tile([NR, 512], F32, tag="rec")
                    nc.vector.reciprocal(rec[lo:hi, :], rows_sb[lo:hi, :])
                    nc.vector.tensor_copy(recip16[lo:hi, :], rec[lo:hi, :])

                return go

            def q_recip_delayed(lo, hi):
                """recip closure + empty pump slots so the PE does not hit
                the dependent normalize MM while the DVE reciprocal runs."""
                return [q_recip(lo, hi)] + [lambda: None] * 3

            def att_S(hc, qn, kc):
                pss = pp_s.tile([128, 1024], F32, tag="pp_s")
                for hb, half in ((0, 0), (64, 1)):
                    nc.tensor.matmul(
                        pss[:, half * 512 : (half + 1) * 512],
                        kT[hb : hb + 64, hc, kc * 128 : (kc + 1) * 128],
                        qT[hb : hb + 64, hc, qn * 512 : (qn + 1) * 512],
                        start=True,
                        stop=True,
                    )
                et = etpool.tile([128, 1024], F16, tag="et")
                nc.scalar.activation(et[:], pss[:], AF.Exp)
                return et

            def att_C(hc, kc, et, pscA, pscB):
                for h, psc, half in ((2 * hc, pscA, 0), (2 * hc + 1, pscB, 1)):
                    nc.tensor.matmul(
                        psc[:],
                        v_sb[:, kc, h * VW : (h + 1) * VW],
                        et[:, half * 512 : (half + 1) * 512],
                        start=(kc == 0),
                        stop=(kc == SC - 1),
                    )

            def att_epi(hc, qn, pscA, pscB, rows_first=False):
                pairs = ((2 * hc, pscA), (2 * hc + 1, pscB))
                if rows_first:
                    # tail-critical: rowsums gate the final reciprocal
                    for h, psc in pairs:
                        rstage = copystage.tile([1, 512], F32, tag="rstage")
                        nc.vector.tensor_copy(rstage[:], psc[64:65, :])
                        r = grow(h * QN + qn)
                        nc.gpsimd.dma_start(rows_sb[r : r + 1, :], rstage[:])
                for h, psc in pairs:
                    hb = (h % 2) * 64
                    dst = ctxU[hb : hb + 64, h // 2, qn * 512 : (qn + 1) * 512]
                    if hb == 0:
                        nc.vector.tensor_copy(dst, psc[0:64, :])
                    else:
                        cst = copystage.tile([64, 512], F16, tag="cst")
                        nc.vector.tensor_copy(cst[:], psc[0:64, :])
                        nc.sync.dma_start(dst, cst[:])
                    if not rows_first:
                        rstage = copystage.tile([1, 512], F32, tag="rstage")
                        nc.vector.tensor_copy(rstage[:], psc[64:65, :])
                        r = grow(h * QN + qn)
                        nc.sync.dma_start(rows_sb[r : r + 1, :], rstage[:])

            prefetched = {}

            def att_pass(hc, qn, nxt=None, pre=None, inline_v=False, cadence=3):
                """Software-pipelined pass: S(kc+1) before C(kc); the next
                pass's S(0) before this pass's last ctx."""
                pscA_t = pp_c.tile([128, 512], F32, tag="pp_c")
                pscB_t = pp_c.tile([128, 512], F32, tag="pp_c")
                pscA, pscB = pscA_t[0:VW, :], pscB_t[0:VW, :]
                if (hc, qn) in prefetched:
                    ets = {0: prefetched.pop((hc, qn))}
                else:
                    ets = {0: att_S(hc, qn, 0)}
                for cl in pre or ():
                    cl()
                for kc in range(SC):
                    if kc + 1 < SC:
                        ets[kc + 1] = att_S(hc, qn, kc + 1)
                    elif nxt is not None:
                        prefetched[nxt] = att_S(nxt[0], nxt[1], 0)
                    att_C(hc, kc, ets.pop(kc), pscA, pscB)
                    if inline_v and kc + 2 < SC:
                        emit_v_grp(kc + 2, 0, 512)
                    if inline_v:
                        # deadline slots for K(0,sn1..3)/Q(0,sn1)
                        if kc in (2, 3, 8, 10):
                            pump(1)
                    elif cadence == 0:
                        # late-start: dependencies of the queued work only
                        # resolve a few us into this pass; stop by kc=13 so
                        # the DVE backlog clears before the tail-epi copies
                        if 5 <= kc < 13:
                            pump(2)
                    elif kc % cadence == cadence - 1:
                        pump(1)
                att_epi(hc, qn, pscA, pscB, rows_first=(nxt is None))

            # ---- startup: minimal deps for attention(0,0) to begin ----
            emit_kq_grp("k", 0, 0)
            emit_kq_grp("q", 0, 0)

            extra.extend(kq_closures(0, skip_first=True))
            extra.extend(kq_closures(1))
            extra.extend(kq_closures(2))
            for sc in range(SC):
                extra.append(lambda sc=sc: emit_v_grp(sc, 512, 256))
            extra.extend(kq_closures(3))
            extra.extend(kq_closures(4))
            extra.extend(kq_closures(5))

            pre00 = [
                lambda: emit_bias_bc(bv_bc, bv_sb, 0, 512),
                lambda: emit_bias_bc(bv_bc, bv_sb, 512, 256),
                lambda: emit_v_grp(0, 0, 512),
                lambda: emit_v_grp(1, 0, 512),
                lambda: emit_bias_bc(bo_bc, bo_sb, 0, 512),
                lambda: emit_bias_bc(bo_bc, bo_sb, 512, 256),
            ]
            passes = [(hc, qn) for hc in range(HC) for qn in range(QN)]
            for i, (hc, qn) in enumerate(passes):
                att_pass(
                    hc,
                    qn,
                    nxt=(passes[i + 1] if i + 1 < len(passes) else None),
                    pre=(pre00 if i == 0 else None),
                    inline_v=(i == 0),
                    cadence=(0 if (hc == 5 and qn == 1) else 1 if hc == 5 else 2),
                )
                if hc == 2 and qn == 1:
                    extra.extend(q_recip_delayed(0, 12))
                    for h in range(6):
                        for q2 in range(QN):
                            extra.append(
                                lambda h=h, q2=q2: normalize_group(h, q2)
                            )
                if hc == 4 and qn == 0:
                    extra.extend(q_recip_delayed(32, 36))
                    for h in range(6, 10):
                        extra.append(lambda h=h: normalize_group(h, 0))
                if hc == 4 and qn == 1:
                    extra.extend(q_recip_delayed(64, 68))
                    for h in range(6, 10):
                        extra.append(lambda h=h: normalize_group(h, 1))
                if hc == 5 and qn == 0:
                    extra.extend(q_recip_delayed(96, 100))
                    for h in (10, 11):
                        extra.append(lambda h=h: normalize_group(h, 0))
                    for qc in range(4):
                        for c0, cw in ((0, 512), (512, 256)):
                            extra.append(
                                lambda qc=qc, c0=c0, cw=cw: emit_out_grp(
                                    qc, c0, cw
                                )
                            )

            # ---- tail. Serial chain: (5,1) rowsums -> reciprocal ->
            # h10/11 qn1 normalize -> mc=5 of out qc4-7. The mc=0..4
            # partials of qc4-6 overlap the reciprocal. ----
            pump(len(extra))

            def out_partial(qc, c0, cw, ps, colofs):
                for mc in range(HC - 1):
                    nc.tensor.matmul(
                        ps[:, colofs : colofs + cw],
                        ctxU[:, mc, qc * 128 : (qc + 1) * 128],
                        wo_sb[:, mc, c0 : c0 + cw],
                        start=(mc == 0),
                        stop=False,
                    )

            def out_finish(qc, c0, cw, ps, colofs):
                nc.tensor.matmul(
                    ps[:, colofs : colofs + cw],
                    ctxU[:, HC - 1, qc * 128 : (qc + 1) * 128],
                    wo_sb[:, HC - 1, c0 : c0 + cw],
                    start=False,
                    stop=True,
                )
                nc.vector.tensor_tensor(
                    stash[:, qc, c0 : c0 + cw],
                    ps[:, colofs : colofs + cw],
                    bo_bc[:, c0 : c0 + cw],
                    ALU.add,
                )

            held = []
            ps40 = pp_mm.tile([128, 512], F32, tag="pp_mm")
            out_partial(4, 0, 512, ps40, 0)
            held.append((4, 0, 512, ps40, 0))
            ps41 = pp_mm.tile([128, 512], F32, tag="pp_mm")
            out_partial(4, 512, 256, ps41, 0)
            held.append((4, 512, 256, ps41, 0))
            ps50 = pp_c.tile([128, 512], F32, tag="pp_c")
            out_partial(5, 0, 512, ps50, 0)
            held.append((5, 0, 512, ps50, 0))
            ps51 = pp_c.tile([128, 512], F32, tag="pp_c")
            out_partial(5, 512, 256, ps51, 0)
            held.append((5, 512, 256, ps51, 0))
            ps6 = pp_s.tile([128, 1024], F32, tag="pp_s")
            out_partial(6, 0, 512, ps6, 0)
            held.append((6, 0, 512, ps6, 0))
            out_partial(6, 512, 256, ps6, 512)
            held.append((6, 512, 256, ps6, 512))

            emit_out_tail(0)
            emit_out_tail(1)

            # reciprocal for h10/11 qn1 (rows 96/97, partition-parallel)
            q_recip(96, 98)()

            # h10/11 qn1 normalize in ONE pp_s tile (both col halves)
            pbt = pp_s.tile([128, 1024], F32, tag="pp_s")
            for h, half in ((10, 0), (11, 1)):
                hb = (h % 2) * 64
                r = grow(h * QN + 1)
                nc.tensor.matmul(
                    pbt[hb : hb + 64, half * 512 : (half + 1) * 512],
                    ident[0:NR, r : r + 1].to_broadcast([NR, 64]),
                    recip16[:],
                    start=True,
                    stop=True,
                )
                sl = ctxU[hb : hb + 64, 5, 512:1024]
                nc.vector.tensor_tensor(
                    sl,
                    sl,
                    pbt[hb : hb + 64, half * 512 : (half + 1) * 512],
                    ALU.mult,
                )
            emit_out_tail(2)
            emit_out_tail(3)

            for qc, c0, cw, ps, colofs in held:
                out_finish(qc, c0, cw, ps, colofs)
            emit_out_tail(4)
            emit_out_tail(5)
            for c0, cw in ((0, 512), (512, 256)):
                emit_out_grp(7, c0, cw)
            emit_out_tail(6)
            emit_out_tail(7)

    split_sync_waits(nc, cap=1)
    return nc


_IDENT = np.eye(128, dtype=np.float16)

_NC_CACHE = None


def _get_nc():
    global _NC_CACHE
    if _NC_CACHE is None:
        _NC_CACHE = build_program()
    return _NC_CACHE


def _install_ntff_hook():
    """The image's antenv lacks axon_hooks; synthesize it so
    run_bass_kernel_spmd(trace=True) can reach the axon NTFF profiler."""
    import types

    if "antenv.axon_hooks" in sys.modules:
        return
    mod = types.ModuleType("antenv.axon_hooks")
    _h = [None]
    mod.set_axon_ntff_profile_hook = lambda h: _h.__setitem__(0, h)
    mod.get_axon_ntff_profile_hook = lambda: _h[0]
    sys.modules["antenv.axon_hooks"] = mod
    import antenv

    antenv.axon_hooks = mod
    from trn_agent_boot.trn_boot import _ntff_profile_via_ctypes

    hook = _ntff_profile_via_ctypes("/opt/axon/libaxon_pjrt.so")
    mod.set_axon_ntff_profile_hook(hook)


def _wsplit(w16, a, b):
    """[768,768] -> [128, 6, 768] (row = c*128+p) split at col a:b."""
    wpc = w16.reshape(HC, 128, H).transpose(1, 0, 2)
    return (
        np.ascontiguousarray(wpc[:, :, :a]),
        np.ascontiguousarray(wpc[:, :, a:b]),
    )


def kernel(
    hidden_states,
    attention_mask,
    Wq,
    bq,
    Wk,
    bk,
    Wv,
    bv,
    Wo,
    bo,
    _trace=False,
):
    from concourse.bass_utils import run_bass_kernel_spmd

    hs = np.asarray(hidden_states, dtype=np.float32)
    f16 = np.float16
    hs16 = hs.astype(f16)
    wq16 = np.asarray(Wq, dtype=np.float32).astype(f16)
    wk16 = np.asarray(Wk, dtype=np.float32).astype(f16)
    wv16 = np.asarray(Wv, dtype=np.float32).astype(f16)
    wo16 = np.asarray(Wo, dtype=np.float32).astype(f16)
    bqf = np.asarray(bq, dtype=np.float32)
    bkf = np.asarray(bk, dtype=np.float32)
    bv16v = np.asarray(bv, dtype=np.float32).astype(f16)
    bo16v = np.asarray(bo, dtype=np.float32).astype(f16)

    wka_h, wkb_h = _wsplit(wk16, 128, H)
    wqa_h, wqb_h = _wsplit(wq16, 128, H)
    wva_h, wvb_h = _wsplit(wv16, 512, H)
    wop_h = np.ascontiguousarray(
        wo16.reshape(HC, 128, H).transpose(1, 0, 2)
    )

    if _trace:
        _install_ntff_hook()
    nc = _get_nc()
    in_maps = []
    for c in range(8):
        b, qh = c // 2, c % 2
        xc = hs16[b] if qh == 0 else np.concatenate(
            [hs16[b, SQ:], hs16[b, :SQ]], axis=0
        )
        # [S, H] -> [128, 6, 2048] with hidden row = c*128+p
        xp = np.ascontiguousarray(xc.T).reshape(HC, 128, S).transpose(1, 0, 2)
        in_maps.append(
            {
                "x0a": np.ascontiguousarray(xp[:, 0:3, 0:512]),
                "x0b": np.ascontiguousarray(xp[:, 3:6, 0:512]),
                "x1": np.ascontiguousarray(xp[:, :, 512:1024]),
                "x2": np.ascontiguousarray(xp[:, :, 1024:1536]),
                "x3": np.ascontiguousarray(xp[:, :, 1536:2048]),
                "ident": _IDENT,
                "wka": wka_h,
                "wkb": wkb_h,
                "wqa": wqa_h,
                "wqb": wqb_h,
                "wva": wva_h,
                "wvb": wvb_h,
                "wop": wop_h,
                "bqf": bqf,
                "bkf": bkf,
                "bv16": bv16v,
                "bo16": bo16v,
            }
        )
    res = run_bass_kernel_spmd(
        nc, in_maps, core_ids=list(range(8)), trace=_trace
    )
    if _trace:
        kernel.last_result = res
    B = hs.shape[0]
    full = np.empty((B, S, H), dtype=np.float32)
    for c in range(8):
        b, qh = c // 2, c % 2
        full[b, qh * SQ : (qh + 1) * SQ] = res.results[c]["out"]
    return full


# revision 50
# speedup vs baseline: 1.0105x; 1.0105x over previous
"""BERT self-attention (B=4, S=2048, H=768, 12 heads) on 8 NeuronCores.

Sharding: core c handles batch b=c//2, query-half qh=c%2 (1024 q rows).
K/V are computed for the full sequence on each core (duplicated across the
2 cores of a batch) so no collectives are needed. Matmul operands are fp16;
accumulation stays fp32 in PSUM.

Host-side prep: x arrives transposed+rotated AND pre-chunked (per-partition
contiguous arrays), weights pre-arranged [128, hc, cols] and split at the
column boundaries the kernel loads first — every input DMA is one
contiguous run per partition (~128 descriptors), so triggers cost ~0.3us
instead of 4-7us and the critical first-scores loads land in ~14us.

Emission is explicitly software-pipelined (the tile scheduler's reorder
window is small, so emission order ~ execution order per engine):
  - scores(kc+1) before ctx(kc); the next pass's scores(0) before this
    pass's last ctx -> the ACT exp stream never waits on the PE queue.
  - deferred work (K/Q proj, V tail, normalize, out-proj) is queued as
    closures and pumped at a fixed cadence into attention windows.
  - warmup matmuls + a dummy exp run during the input DMAs (HAM clock
    ramp + ACT exp-table prefetch), made load-bearing via ones = exp(0).
  - tail: out-proj mc=0..4 partial sums held in psum overlap the final
    reciprocal; only the mc=5 step waits on the last normalize; gelu is
    batched after one ACT table switch.
"""

import sys

sys.path.insert(0, "/opt/trn_rl_repo")

import numpy as np

import concourse.bass as bass
import concourse.tile as tile
import concourse.mybir as mybir

F16 = mybir.dt.float16
F32 = mybir.dt.float32
AF = mybir.ActivationFunctionType
ALU = mybir.AluOpType

S = 2048
SQ = 1024
H = 768
NH = 12
DH = 64
HC = H // 128  # 6
SC = S // 128  # 16
QC = SQ // 128  # 8
VW = DH + 1  # 65
QN = SQ // 512  # 2
NG = NH * QN  # 24


def split_sync_waits(nc, cap=1):
    """Walrus here rejects instructions carrying more than ~1 sync wait.
    Move excess waits onto same-engine NoOps inserted just before."""
    n = 0
    for b in nc.m.functions[0].blocks:
        out = []
        for inst in b.instructions:
            si = inst.sync_info
            waits = list(si.on_wait) if si is not None and si.on_wait else []
            if len(waits) > cap:
                extra, keep = waits[:-cap], waits[-cap:]
                for i in range(0, len(extra), cap):
                    nop = mybir.InstNoOp(
                        name=f"wsplit-{n}",
                        engine=inst.engine,
                        sync_info=mybir.SyncInfo(
                            on_wait=extra[i : i + cap], on_update=[]
                        ),
                    )
                    n += 1
                    out.append(nop)
                si.on_wait = keep
            out.append(inst)
        b.instructions[:] = out
    return n


def build_program():
    from collections import deque

    nc = bass.Bass()
    # x chunks: [128, ic, 512] per-partition-contiguous
    x0a = nc.declare_dram_parameter("x0a", [128, 3, 512], F16, isOutput=False)
    x0b = nc.declare_dram_parameter("x0b", [128, 3, 512], F16, isOutput=False)
    x1 = nc.declare_dram_parameter("x1", [128, HC, 512], F16, isOutput=False)
    x2 = nc.declare_dram_parameter("x2", [128, HC, 512], F16, isOutput=False)
    x3 = nc.declare_dram_parameter("x3", [128, HC, 512], F16, isOutput=False)
    ident_in = nc.declare_dram_parameter("ident", [128, 128], F16, isOutput=False)
    wka = nc.declare_dram_parameter("wka", [128, HC, 128], F16, isOutput=False)
    wkb = nc.declare_dram_parameter("wkb", [128, HC, 640], F16, isOutput=False)
    wqa = nc.declare_dram_parameter("wqa", [128, HC, 128], F16, isOutput=False)
    wqb = nc.declare_dram_parameter("wqb", [128, HC, 640], F16, isOutput=False)
    wva = nc.declare_dram_parameter("wva", [128, HC, 512], F16, isOutput=False)
    wvb = nc.declare_dram_parameter("wvb", [128, HC, 256], F16, isOutput=False)
    wop = nc.declare_dram_parameter("wop", [128, HC, H], F16, isOutput=False)
    bqf = nc.declare_dram_parameter("bqf", [H], F32, isOutput=False)
    bkf = nc.declare_dram_parameter("bkf", [H], F32, isOutput=False)
    bv16 = nc.declare_dram_parameter("bv16", [H], F16, isOutput=False)
    bo16 = nc.declare_dram_parameter("bo16", [H], F16, isOutput=False)
    out = nc.declare_dram_parameter("out", [SQ, H], F32, isOutput=True)

    with tile.TileContext(nc) as tc:
        from contextlib import ExitStack

        with ExitStack() as ctx:
            consts = ctx.enter_context(tc.tile_pool(name="consts", bufs=1))
            wpool = ctx.enter_context(tc.tile_pool(name="wpool", bufs=1))
            big = ctx.enter_context(tc.tile_pool(name="big", bufs=1))
            copystage = ctx.enter_context(tc.tile_pool(name="copystage", bufs=3))
            etpool = ctx.enter_context(tc.tile_pool(name="etpool", bufs=8))
            recpool = ctx.enter_context(tc.tile_pool(name="recpool", bufs=1))
            outstage = ctx.enter_context(tc.tile_pool(name="outstage", bufs=2))
            pp_mm = ctx.enter_context(
                tc.tile_pool(name="pp_mm", bufs=2, space="PSUM")
            )

            # ---- constants (fast queues; gpsimd preamble is ~7us) ----
            ident = consts.tile([128, 128], F16, tag="ident")
            nc.sync.dma_start(ident[:], ident_in[:])
            junk = consts.tile([128, 512], F32, tag="junk")
            nc.vector.memset(junk[:], 0.0)
            # ones via exp(0): ACT exp-table load becomes load-bearing+early
            ones16 = consts.tile([128, 512], F16, tag="ones16")
            nc.scalar.activation(ones16[:], junk[:], AF.Exp)

            # ---- HAM warmup, load-bearing: onesW = ident.T @ ones16 ----
            onesW = consts.tile([128, 512], F16, tag="onesW")
            with tc.tile_pool(name="pp_warm", bufs=1, space="PSUM") as pp_warm:
                warm = pp_warm.tile([128, 512], F32, tag="warm")
                for _ in range(32):
                    nc.tensor.matmul(
                        warm[:, 0:128],
                        ident[:],
                        ones16[:, 0:128],
                        start=True,
                        stop=True,
                    )
                nc.tensor.matmul(
                    warm[:], ident[:], ones16[:], start=True, stop=True
                )
                nc.vector.tensor_copy(onesW[:], warm[:])

            # ---- SBUF tiles for weights / x ----
            wka_sb = wpool.tile([128, HC, 128], F16, tag="wka")
            wkb_sb = wpool.tile([128, HC, 640], F16, tag="wkb")
            wqa_sb = wpool.tile([128, HC, 128], F16, tag="wqa")
            wqb_sb = wpool.tile([128, HC, 640], F16, tag="wqb")
            wva_sb = wpool.tile([128, HC, 512], F16, tag="wva")
            wvb_sb = wpool.tile([128, HC, 256], F16, tag="wvb")
            wo_sb = wpool.tile([128, HC, H], F16, tag="wo")
            bq_sb = wpool.tile([128, HC], F32, tag="bq")
            bk_sb = wpool.tile([128, HC], F32, tag="bk")
            bv_sb = wpool.tile([1, H], F16, tag="bv")
            bo_sb = wpool.tile([1, H], F16, tag="bo")
            # x as [128, chunk, ic, 512] so chunk loads are contiguous
            xTc = big.tile([128, 4, HC, 512], F16, tag="xTc")
            v_sb = big.tile([128, SC, NH * VW], F16, tag="v")
            v_heads = v_sb[:].rearrange("p s (h c) -> p s h c", c=VW)

            # ---- DMA queue plan. Within a queue transfers serialize, so
            # order IS the priority. Critical for first scores: x0a+x0b,
            # wka, wqa (~1.1MB). Everything else queues behind. ----
            # sync:   ident, x0a, x2, x3
            # scalar: x0b, wka, wqa, wva, wkb, wqb, wvb, wop
            # gpsimd: v-ones memset, x1, biases, tail memsets
            nc.gpsimd.memset(v_heads[:, :, :, DH], 1.0)
            nc.scalar.dma_start(wka_sb[:], wka[:])
            nc.scalar.dma_start(wqa_sb[:], wqa[:])
            # x chunk0 split per-ic: the K(0,sn0) accumulation chain
            # starts when ic0 lands (~1us) instead of the whole chunk
            for i in range(3):
                nc.sync.dma_start(xTc[:, 0, i, :], x0a[:, i, :])
            for i in range(3):
                nc.scalar.dma_start(xTc[:, 0, 3 + i, :], x0b[:, i, :])
            nc.gpsimd.dma_start(xTc[:, 1, 0:3, :], x1[:, 0:3, :])
            nc.sync.dma_start(xTc[:, 1, 3:6, :], x1[:, 3:6, :])
            nc.scalar.dma_start(wva_sb[:], wva[:])
            nc.sync.dma_start(xTc[:, 2, :, :], x2[:])
            nc.sync.dma_start(xTc[:, 3, :, :], x3[:])
            nc.gpsimd.dma_start(bq_sb[:], bqf.rearrange("(c p) -> p c", p=128))
            nc.gpsimd.dma_start(bk_sb[:], bkf.rearrange("(c p) -> p c", p=128))
            nc.gpsimd.dma_start(bv_sb[:], bv16[None, :])
            nc.gpsimd.dma_start(bo_sb[:], bo16[None, :])
            nc.scalar.dma_start(wkb_sb[:], wkb[:])
            nc.scalar.dma_start(wqb_sb[:], wqb[:])
            nc.scalar.dma_start(wvb_sb[:], wvb[:])
            nc.scalar.dma_start(wo_sb[:], wop[:])

            def kq_w(proj, ic, hc):
                if hc == 0:
                    return (wka_sb if proj == "k" else wqa_sb)[:, ic, :]
                t = wkb_sb if proj == "k" else wqb_sb
                return t[:, ic, (hc - 1) * 128 : hc * 128]

            def x_at(ic, s0, sw):
                """xTc slice for seq [s0, s0+sw); must stay in one chunk."""
                ch = s0 // 512
                o = s0 % 512
                return xTc[:, ch, ic, o : o + sw]

            # ---- bias broadcast across partitions (K=1 matmul on onesW) --
            bv_bc = wpool.tile([128, H], F32, tag="bv_bc")
            bo_bc = wpool.tile([128, H], F32, tag="bo_bc")

            def emit_bias_bc(bc, bsb, c0, cw):
                ps = pp_mm.tile([128, 512], F32, tag="pp_mm")
                nc.tensor.matmul(
                    ps[:, :cw],
                    onesW[0:1, 0:128],
                    bsb[:, c0 : c0 + cw],
                    start=True,
                    stop=True,
                )
                nc.vector.tensor_copy(bc[:, c0 : c0 + cw], ps[:, :cw])

            def emit_v_grp(sc, c0, cw):
                """V columns c0:c0+cw for seq chunk sc (c0 head-aligned:
                0:512 = heads 0-7, 512:768 = heads 8-11)."""
                ps = pp_mm.tile([128, 512], F32, tag="pp_mm")
                wsrc = wva_sb if c0 == 0 else wvb_sb
                for ic in range(HC):
                    nc.tensor.matmul(
                        ps[:, :cw],
                        x_at(ic, sc * 128, 128),
                        wsrc[:, ic, :],
                        start=(ic == 0),
                        stop=(ic == HC - 1),
                    )
                h0 = c0 // DH
                nhh = cw // DH
                nc.vector.scalar_tensor_tensor(
                    v_heads[:, sc, h0 : h0 + nhh, 0:DH],
                    ps[:, :cw].rearrange("p (h c) -> p h c", c=DH),
                    1.0,
                    bv_bc[:, c0 : c0 + cw].rearrange("p (h c) -> p h c", c=DH),
                    ALU.mult,
                    ALU.add,
                )

            pp_s = ctx.enter_context(tc.tile_pool(name="pp_s", bufs=2, space="PSUM"))
            pp_c = ctx.enter_context(tc.tile_pool(name="pp_c", bufs=2, space="PSUM"))

            out_t = out.rearrange("(n p) h -> n p h", p=128)

            kT = big.tile([128, HC, S], F16, tag="kT")
            qT = big.tile([128, HC, SQ], F16, tag="qT")
            ctxU = big.tile([128, HC, SQ], F16, tag="ctxU")
            stash = big.tile([128, QC, H], F32, tag="stash")
            # group row map: bases must be 0/32/64/96 for reciprocal
            # batches: rows 0-11 = heads 0-5 both qn; 32-35 = h6-9 qn0;
            # 64-67 = h6-9 qn1; 96-97 = h10-11 qn1 (tail); 98-99 = h10-11
            # qn0 (computed in the [96:100) batch after (5,0); its rows
            # 96/97 are then still padding, harmless).
            NR = 100
            rows_sb = big.tile([NR, 512], F32, tag="rows")
            recip16 = big.tile([NR, 512], F16, tag="recip16")
            nc.gpsimd.memset(recip16[:], 0.0)
            nc.gpsimd.memset(rows_sb[:], 1.0)

            def grow(g):
                h, qn = g // QN, g % QN
                if h < NH // 2:
                    return g
                if qn == 0:
                    return 32 + (h - NH // 2) if h < 10 else 98 + (h - 10)
                return 64 + (h - NH // 2) if h < 10 else 96 + (h - 10)

            def emit_kq_grp(proj, hc, sn):
                """One (proj, hc, sn) group: 6 accum MMs + bias add."""
                ps = pp_mm.tile([128, 512], F32, tag="pp_mm")
                for ic in range(HC):
                    nc.tensor.matmul(
                        ps[:],
                        kq_w(proj, ic, hc),
                        xTc[:, sn, ic, :],
                        start=(ic == 0),
                        stop=(ic == HC - 1),
                    )
                if proj == "k":
                    nc.vector.tensor_scalar_add(
                        kT[:, hc, sn * 512 : (sn + 1) * 512],
                        ps[:],
                        bk_sb[:, hc : hc + 1],
                    )
                else:
                    nc.vector.tensor_scalar(
                        qT[:, hc, sn * 512 : (sn + 1) * 512],
                        ps[:],
                        bq_sb[:, hc : hc + 1],
                        0.125,
                        ALU.add,
                        ALU.mult,
                    )

            def kq_closures(hc, skip_first=False):
                cl = []
                if not skip_first:
                    cl.append(lambda hc=hc: emit_kq_grp("q", hc, 0))
                for sn in range(4):
                    if skip_first and sn == 0:
                        continue
                    cl.append(lambda hc=hc, sn=sn: emit_kq_grp("k", hc, sn))
                cl.append(lambda hc=hc: emit_kq_grp("q", hc, 1))
                return cl

            def normalize_group(h, qn):
                hb = (h % 2) * 64
                r = grow(h * QN + qn)
                pb = pp_mm.tile([128, 512], F32, tag="pp_mm")
                nc.tensor.matmul(
                    pb[hb : hb + 64, :],
                    ident[0:NR, r : r + 1].to_broadcast([NR, 64]),
                    recip16[:],
                    start=True,
                    stop=True,
                )
                sl = ctxU[hb : hb + 64, h // 2, qn * 512 : (qn + 1) * 512]
                nc.vector.tensor_tensor(sl, sl, pb[hb : hb + 64, :], ALU.mult)

            def emit_out_grp(qc, c0, cw):
                ps = pp_mm.tile([128, 512], F32, tag="pp_mm")
                for mc in range(HC):
                    nc.tensor.matmul(
                        ps[:, :cw],
                        ctxU[:, mc, qc * 128 : (qc + 1) * 128],
                        wo_sb[:, mc, c0 : c0 + cw],
                        start=(mc == 0),
                        stop=(mc == HC - 1),
                    )
                nc.vector.tensor_tensor(
                    stash[:, qc, c0 : c0 + cw],
                    ps[:, :cw],
                    bo_bc[:, c0 : c0 + cw],
                    ALU.add,
                )

            def emit_out_tail(qc):
                ost = outstage.tile([128, H], F32, tag="ost")
                nc.scalar.activation(ost[:], stash[:, qc, :], AF.Gelu)
                q = nc.sync if qc % 2 == 0 else nc.scalar
                q.dma_start(out_t[qc][:, :], ost[:])

            # ---- deferred-work queue, pumped into attention windows ----
            extra = deque()

            def pump(n=1):
                for _ in range(n):
                    if not extra:
                        return
                    extra.popleft()()

            def q_recip(lo, hi):
                def go():
                    rec = recpool.tile([NR, 512], F32, tag="rec")
                    nc.vector.reciprocal(rec[lo:hi, :], rows_sb[lo:hi, :])
                    nc.vector.tensor_copy(recip16[lo:hi, :], rec[lo:hi, :])

                return go

            def q_recip_delayed(lo, hi):
                """recip closure + empty pump slots so the PE does not hit
                the dependent normalize MM while the DVE reciprocal runs."""
                return [q_recip(lo, hi)] + [lambda: None] * 3

            def att_S(hc, qn, kc):
                pss = pp_s.tile([128, 1024], F32, tag="pp_s")
                for hb, half in ((0, 0), (64, 1)):
                    nc.tensor.matmul(
                        pss[:, half * 512 : (half + 1) * 512],
                        kT[hb : hb + 64, hc, kc * 128 : (kc + 1) * 128],
                        qT[hb : hb + 64, hc, qn * 512 : (qn + 1) * 512],
                        start=True,
                        stop=True,
                    )
                et = etpool.tile([128, 1024], F16, tag="et")
                nc.scalar.activation(et[:], pss[:], AF.Exp)
                return et

            def att_C(hc, kc, et, pscA, pscB):
                for h, psc, half in ((2 * hc, pscA, 0), (2 * hc + 1, pscB, 1)):
                    nc.tensor.matmul(
                        psc[:],
                        v_sb[:, kc, h * VW : (h + 1) * VW],
                        et[:, half * 512 : (half + 1) * 512],
                        start=(kc == 0),
                        stop=(kc == SC - 1),
                    )

            def att_epi(hc, qn, pscA, pscB, rows_first=False):
                pairs = ((2 * hc, pscA), (2 * hc + 1, pscB))
                if rows_first:
                    # tail-critical: rowsums gate the final reciprocal
                    for h, psc in pairs:
                        rstage = copystage.tile([1, 512], F32, tag="rstage")
                        nc.vector.tensor_copy(rstage[:], psc[64:65, :])
                        r = grow(h * QN + qn)
                        nc.gpsimd.dma_start(rows_sb[r : r + 1, :], rstage[:])
                for h, psc in pairs:
                    hb = (h % 2) * 64
                    dst = ctxU[hb : hb + 64, h // 2, qn * 512 : (qn + 1) * 512]
                    if hb == 0:
                        nc.vector.tensor_copy(dst, psc[0:64, :])
                    else:
                        cst = copystage.tile([64, 512], F16, tag="cst")
                        nc.vector.tensor_copy(cst[:], psc[0:64, :])
                        nc.sync.dma_start(dst, cst[:])
                    if not rows_first:
                        rstage = copystage.tile([1, 512], F32, tag="rstage")
                        nc.vector.tensor_copy(rstage[:], psc[64:65, :])
                        r = grow(h * QN + qn)
                        nc.sync.dma_start(rows_sb[r : r + 1, :], rstage[:])

            prefetched = {}

            def att_pass(hc, qn, nxt=None, pre=None, inline_v=False, cadence=3):
                """Software-pipelined pass: S(kc+1) before C(kc); the next
                pass's S(0) before this pass's last ctx."""
                pscA_t = pp_c.tile([128, 512], F32, tag="pp_c")
                pscB_t = pp_c.tile([128, 512], F32, tag="pp_c")
                pscA, pscB = pscA_t[0:VW, :], pscB_t[0:VW, :]
                if (hc, qn) in prefetched:
                    ets = {0: prefetched.pop((hc, qn))}
                else:
                    ets = {0: att_S(hc, qn, 0)}
                for cl in pre or ():
                    cl()
                for kc in range(SC):
                    if kc + 1 < SC:
                        ets[kc + 1] = att_S(hc, qn, kc + 1)
                    elif nxt is not None:
                        prefetched[nxt] = att_S(nxt[0], nxt[1], 0)
                    att_C(hc, kc, ets.pop(kc), pscA, pscB)
                    if inline_v and kc + 2 < SC:
                        emit_v_grp(kc + 2, 0, 512)
                    if inline_v:
                        # deadline slots for K(0,sn1..3)/Q(0,sn1)
                        if kc in (2, 3, 8, 10):
                            pump(1)
                    elif cadence == 0:
                        # late-start: dependencies of the queued work only
                        # resolve a few us into this pass; stop by kc=12 so
                        # the DVE backlog clears before the tail-epi copies
                        if 4 <= kc < 12:
                            pump(2)
                    elif kc % cadence == cadence - 1:
                        pump(1)
                att_epi(hc, qn, pscA, pscB, rows_first=(nxt is None))

            # ---- startup: minimal deps for attention(0,0) to begin ----
            emit_kq_grp("k", 0, 0)
            emit_kq_grp("q", 0, 0)

            extra.extend(kq_closures(0, skip_first=True))
            extra.extend(kq_closures(1))
            extra.extend(kq_closures(2))
            for sc in range(SC):
                extra.append(lambda sc=sc: emit_v_grp(sc, 512, 256))
            extra.extend(kq_closures(3))
            extra.extend(kq_closures(4))
            extra.extend(kq_closures(5))

            pre00 = [
                lambda: emit_bias_bc(bv_bc, bv_sb, 0, 512),
                lambda: emit_bias_bc(bv_bc, bv_sb, 512, 256),
                lambda: emit_v_grp(0, 0, 512),
                lambda: emit_v_grp(1, 0, 512),
                lambda: emit_bias_bc(bo_bc, bo_sb, 0, 512),
                lambda: emit_bias_bc(bo_bc, bo_sb, 512, 256),
            ]
            passes = [(hc, qn) for hc in range(HC) for qn in range(QN)]
            for i, (hc, qn) in enumerate(passes):
                att_pass(
                    hc,
                    qn,
                    nxt=(passes[i + 1] if i + 1 < len(passes) else None),
                    pre=(pre00 if i == 0 else None),
                    inline_v=(i == 0),
                    cadence=(0 if (hc == 5 and qn == 1) else 1 if hc == 5 else 2),
                )
                if hc == 2 and qn == 1:
                    extra.extend(q_recip_delayed(0, 12))
                    for h in range(6):
                        for q2 in range(QN):
                            extra.append(
                                lambda h=h, q2=q2: normalize_group(h, q2)
                            )
                if hc == 4 and qn == 0:
                    extra.extend(q_recip_delayed(32, 36))
                    for h in range(6, 10):
                        extra.append(lambda h=h: normalize_group(h, 0))
                if hc == 4 and qn == 1:
                    extra.extend(q_recip_delayed(64, 68))
                    for h in range(6, 10):
                        extra.append(lambda h=h: normalize_group(h, 1))
                if hc == 5 and qn == 0:
                    extra.extend(q_recip_delayed(96, 100))
                    for h in (10, 11):
                        extra.append(lambda h=h: normalize_group(h, 0))
                    for qc in range(4):
                        for c0, cw in ((0, 512), (512, 256)):
                            extra.append(
                                lambda qc=qc, c0=c0, cw=cw: emit_out_grp(
                                    qc, c0, cw
                                )
                            )

            # ---- tail. Serial chain: (5,1) rowsums -> reciprocal ->
            # h10/11 qn1 normalize -> mc=5 of out qc4-7. The mc=0..4
            # partials of qc4-6 overlap the reciprocal. ----
            pump(len(extra))

            def out_partial(qc, c0, cw, ps, colofs):
                for mc in range(HC - 1):
                    nc.tensor.matmul(
                        ps[:, colofs : colofs + cw],
                        ctxU[:, mc, qc * 128 : (qc + 1) * 128],
                        wo_sb[:, mc, c0 : c0 + cw],
                        start=(mc == 0),
                        stop=False,
                    )

            def out_finish(qc, c0, cw, ps, colofs):
                nc.tensor.matmul(
                    ps[:, colofs : colofs + cw],
                    ctxU[:, HC - 1, qc * 128 : (qc + 1) * 128],
                    wo_sb[:, HC - 1, c0 : c0 + cw],
                    start=False,
                    stop=True,
                )
                nc.vector.tensor_tensor(
                    stash[:, qc, c0 : c0 + cw],
                    ps[:, colofs : colofs + cw],
                    bo_bc[:, c0 : c0 + cw],
                    ALU.add,
                )

            held = []
            ps40 = pp_mm.tile([128, 512], F32, tag="pp_mm")
            out_partial(4, 0, 512, ps40, 0)
            held.append((4, 0, 512, ps40, 0))
            ps41 = pp_mm.tile([128, 512], F32, tag="pp_mm")
            out_partial(4, 512, 256, ps41, 0)
            held.append((4, 512, 256, ps41, 0))
            ps50 = pp_c.tile([128, 512], F32, tag="pp_c")
            out_partial(5, 0, 512, ps50, 0)
            held.append((5, 0, 512, ps50, 0))
            ps51 = pp_c.tile([128, 512], F32, tag="pp_c")
            out_partial(5, 512, 256, ps51, 0)
            held.append((5, 512, 256, ps51, 0))
            ps6 = pp_s.tile([128, 1024], F32, tag="pp_s")
            out_partial(6, 0, 512, ps6, 0)
            held.append((6, 0, 512, ps6, 0))
            out_partial(6, 512, 256, ps6, 512)
            held.append((6, 512, 256, ps6, 512))

            emit_out_tail(0)
            emit_out_tail(1)

            # reciprocal for h10/11 qn1 (rows 96/97, partition-parallel)
            q_recip(96, 98)()

            # h10/11 qn1 normalize in ONE pp_s tile (both col halves)
            pbt = pp_s.tile([128, 1024], F32, tag="pp_s")
            for h, half in ((10, 0), (11, 1)):
                hb = (h % 2) * 64
                r = grow(h * QN + 1)
                nc.tensor.matmul(
                    pbt[hb : hb + 64, half * 512 : (half + 1) * 512],
                    ident[0:NR, r : r + 1].to_broadcast([NR, 64]),
                    recip16[:],
                    start=True,
                    stop=True,
                )
                sl = ctxU[hb : hb + 64, 5, 512:1024]
                nc.vector.tensor_tensor(
                    sl,
                    sl,
                    pbt[hb : hb + 64, half * 512 : (half + 1) * 512],
                    ALU.mult,
                )
            emit_out_tail(2)
            emit_out_tail(3)

            for qc, c0, cw, ps, colofs in held:
                out_finish(qc, c0, cw, ps, colofs)
            emit_out_tail(4)
            emit_out_tail(5)
            for c0, cw in ((0, 512), (512, 256)):
                emit_out_grp(7, c0, cw)
            emit_out_tail(6)
            emit_out_tail(7)

    split_sync_waits(nc, cap=1)
    return nc


_IDENT = np.eye(128, dtype=np.float16)

_NC_CACHE = None


def _get_nc():
    global _NC_CACHE
    if _NC_CACHE is None:
        _NC_CACHE = build_program()
    return _NC_CACHE


def _install_ntff_hook():
    """The image's antenv lacks axon_hooks; synthesize it so
    run_bass_kernel_spmd(trace=True) can reach the axon NTFF profiler."""
    import types

    if "antenv.axon_hooks" in sys.modules:
        return
    mod = types.ModuleType("antenv.axon_hooks")
    _h = [None]
    mod.set_axon_ntff_profile_hook = lambda h: _h.__setitem__(0, h)
    mod.get_axon_ntff_profile_hook = lambda: _h[0]
    sys.modules["antenv.axon_hooks"] = mod
    import antenv

    antenv.axon_hooks = mod
    from trn_agent_boot.trn_boot import _ntff_profile_via_ctypes

    hook = _ntff_profile_via_ctypes("/opt/axon/libaxon_pjrt.so")
    mod.set_axon_ntff_profile_hook(hook)


def _wsplit(w16, a, b):
    """[768,768] -> [128, 6, 768] (row = c*128+p) split at col a:b."""
    wpc = w16.reshape(HC, 128, H).transpose(1, 0, 2)
    return (
        np.ascontiguousarray(wpc[:, :, :a]),
        np.ascontiguousarray(wpc[:, :, a:b]),
    )


def kernel(
    hidden_states,
    attention_mask,
    Wq,
    bq,
    Wk,
    bk,
    Wv,
    bv,
    Wo,
    bo,
    _trace=False,
):
    from concourse.bass_utils import run_bass_kernel_spmd

    hs = np.asarray(hidden_states, dtype=np.float32)
    f16 = np.float16
    hs16 = hs.astype(f16)
    wq16 = np.asarray(Wq, dtype=np.float32).astype(f16)
    wk16 = np.asarray(Wk, dtype=np.float32).astype(f16)
    wv16 = np.asarray(Wv, dtype=np.float32).astype(f16)
    wo16 = np.asarray(Wo, dtype=np.float32).astype(f16)
    bqf = np.asarray(bq, dtype=np.float32)
    bkf = np.asarray(bk, dtype=np.float32)
    bv16v = np.asarray(bv, dtype=np.float32).astype(f16)
    bo16v = np.asarray(bo, dtype=np.float32).astype(f16)

    wka_h, wkb_h = _wsplit(wk16, 128, H)
    wqa_h, wqb_h = _wsplit(wq16, 128, H)
    wva_h, wvb_h = _wsplit(wv16, 512, H)
    wop_h = np.ascontiguousarray(
        wo16.reshape(HC, 128, H).transpose(1, 0, 2)
    )

    if _trace:
        _install_ntff_hook()
    nc = _get_nc()
    in_maps = []
    for c in range(8):
        b, qh = c // 2, c % 2
        xc = hs16[b] if qh == 0 else np.concatenate(
            [hs16[b, SQ:], hs16[b, :SQ]], axis=0
        )
        # [S, H] -> [128, 6, 2048] with hidden row = c*128+p
        xp = np.ascontiguousarray(xc.T).reshape(HC, 128, S).transpose(1, 0, 2)
        in_maps.append(
            {
                "x0a": np.ascontiguousarray(xp[:, 0:3, 0:512]),
                "x0b": np.ascontiguousarray(xp[:, 3:6, 0:512]),
                "x1": np.ascontiguousarray(xp[:, :, 512:1024]),
                "x2": np.ascontiguousarray(xp[:, :, 1024:1536]),
                "x3": np.ascontiguousarray(xp[:, :, 1536:2048]),
                "ident": _IDENT,
                "wka": wka_h,
                "wkb": wkb_h,
                "wqa": wqa_h,
                "wqb": wqb_h,
                "wva": wva_h,
                "wvb": wvb_h,
                "wop": wop_h,
                "bqf": bqf,
                "bkf": bkf,
                "bv16": bv16v,
                "bo16": bo16v,
            }
        )
    res = run_bass_kernel_spmd(
        nc, in_maps, core_ids=list(range(8)), trace=_trace
    )
    if _trace:
        kernel.last_result = res
    B = hs.shape[0]
    full = np.empty((B, S, H), dtype=np.float32)
    for c in range(8):
        b, qh = c // 2, c % 2
        full[b, qh * SQ : (qh + 1) * SQ] = res.results[c]["out"]
    return full
